# revision 21
# baseline (speedup 1.0000x reference)
"""Trainium2 Bass kernel for Llama GQA self-attention (b=2, s=2048, E=4096,
32 Q heads / 8 KV heads, RoPE, causal) sharded tensor-parallel over 8
NeuronCores (one KV-head group per core).

v2: all-bf16 data path, SBUF-resident qkv (no DRAM roundtrip), chunked
startup loads, RoPE with sign-folded sin table (GpSimd half-copies + DVE
bf16 mults), batch-0 RoPE hidden under phase 1, paired exp instructions
([128,1024] PSUM tiles), phase-3 with batched 1MB stores on the GpSimd
DMA queue, bf16 output partials summed on host.

Per-core pipeline:
  phase 1: qkv_sb[6][128, 4096] (bf16, SBUF) = w_qkvT.T @ xT
           (batch-0 token blocks first; RoPE b0 + v-transpose b0 emitted
           mid-loop so DVE/GpSimd work hides under phase-1 PE)
  phase 2: per (batch, head): scoresT[kt, qt] = kT.T @ qT -> exp
           (ScalarE, paired tiles) -> tri-mask diag (DVE) ->
           AV: out[qt, 129] = expT.T @ [v | 1] -> normalize -> transpose
           -> aoT bf16
  phase 3: out[tok, 4096] = sum_et aoT[et].T @ w_outT  (bf16, batched
           1MB stores; first half interleaved into batch-1 attention)
Host: sum the 8 partial outputs.
"""

import os
import sys

import numpy as np

for _p in ("/opt/trn_rl_repo", "/root/.axon_site/_ro/trn_rl_repo"):
    if os.path.isdir(_p) and _p not in sys.path:
        sys.path.append(_p)

import ml_dtypes  # noqa: E402

import concourse.bass as bass  # noqa: E402
import concourse.mybir as mybir  # noqa: E402
import concourse.tile as tile  # noqa: E402
from concourse import bacc  # noqa: E402
from concourse.bass_utils import run_bass_kernel_spmd  # noqa: E402

F32 = mybir.dt.float32
BF16 = mybir.dt.bfloat16
NPBF16 = ml_dtypes.bfloat16

EMBED = 4096
N_HEADS = 32
N_KV = 8
HEAD_DIM = 128
Q_PER_KV = 4
B = 2
S = 2048
TOK = B * S            # 4096
NCORES = 8
ODIM = 768             # per-core qkv rows: 4 q heads + k + v
SCALE = HEAD_DIM ** -0.5
ROPE_BASE = 10000.0

TB = 256               # phase-1 token block (moving N)
NB = TOK // TB         # 16
ET1 = EMBED // 128     # 32 contraction tiles
NM1 = ODIM // 128      # 6 output row tiles

ALU = mybir.AluOpType
ACTF = mybir.ActivationFunctionType


def _emit(nc, tc, h):
    with (
        tc.tile_pool(name="cp", bufs=1) as cp,
        tc.tile_pool(name="psum", bufs=1, space="PSUM") as pp,
        tc.tile_pool(name="ep", bufs=1) as ep,
    ):
        tri = cp.tile([128, 128], BF16, tag="tri")
        idbf = cp.tile([128, 128], BF16, tag="idbf")

        qkv = [
            cp.tile([128, TOK], BF16, tag=f"qkv{m}", name=f"qkv{m}")
            for m in range(NM1)
        ]
        aoT = [
            cp.tile([128, TOK], BF16, tag=f"aoT{hh}", name=f"aoT{hh}")
            for hh in range(4)
        ]

        rot = [[None] * 5 for _ in range(B)]
        vaug = [[None] * 16 for _ in range(B)]
        wo = [None] * 8
        tabs = {}

        def emit_vaug(b):
            # v slice is bf16 in SBUF already: transpose tokens<->dims
            for vt in range(16):
                pst = pp.tile([128, 128], BF16, tag="tr", bufs=1)
                nc.tensor.matmul(
                    pst,
                    lhsT=qkv[5][:, b * S + vt * 128 : b * S + (vt + 1) * 128],
                    rhs=idbf,
                    is_transpose=True,
                )
                va = ep.tile([128, 132], BF16, tag="vaug", bufs=34)
                nc.gpsimd.memset(va[:, 128:129], 1.0)
                nc.vector.tensor_copy(va[:, 0:128], pst)
                vaug[b][vt] = va

        def emit_rope_slice(b, hs):
            # half-layout RoPE, in place: rows 0:64 pair-elem x1, rows
            # 64:128 x2; rt = [x2*-s; x1*+s]; src = src*cos + rt
            sl = qkv[hs][:, b * S : (b + 1) * S]
            rt = ep.tile([128, S], BF16, tag="rt", bufs=1)
            nc.vector.tensor_copy(rt[0:64, :], sl[64:128, :])
            nc.vector.tensor_copy(rt[64:128, :], sl[0:64, :])
            nc.vector.tensor_mul(rt, rt, tabs["sin"])
            nc.vector.tensor_mul(sl, sl, tabs["cos"])
            nc.vector.tensor_tensor(sl, sl, rt, ALU.add)
            rot[b][hs] = sl

        def attention_head(b, hh, u_cb=None, j_cb=None):
            kr = rot[b][4]
            qr = rot[b][hh]
            for j in range(4):
                nt = 4 * j + 4
                ets = {}
                for p0 in range(0, nt, 2):
                    ps2 = pp.tile([128, 1024], F32, tag="sc2", bufs=2)
                    lo = None
                    for t in (p0, p0 + 1):
                        off = (t - p0) * 512
                        c0 = 128 * (t - 4 * j) if t >= 4 * j else 0
                        if lo is None:
                            lo = off + c0
                        nc.tensor.matmul(
                            ps2[:, off + c0 : off + 512],
                            lhsT=kr[:, t * 128 : (t + 1) * 128],
                            rhs=qr[:, j * 512 + c0 : (j + 1) * 512],
                            start=True,
                            stop=True,
                        )
                    et2 = ep.tile([128, 1024], BF16, tag="exp", bufs=9)
                    if p0 + 1 >= 4 * j:
                        # diag pair: written ranges are disjoint; exp
                        # each segment to avoid uninitialized psum
                        for t in (p0, p0 + 1):
                            off = (t - p0) * 512
                            c0 = 128 * (t - 4 * j) if t >= 4 * j else 0
                            nc.scalar.activation(
                                et2[:, off + c0 : off + 512],
                                ps2[:, off + c0 : off + 512],
                                ACTF.Exp, scale=SCALE,
                            )
                    else:
                        nc.scalar.activation(
                            et2[:, lo:1024], ps2[:, lo:1024], ACTF.Exp,
                            scale=SCALE,
                        )
                    for t in (p0, p0 + 1):
                        if t >= 4 * j:
                            off = (t - p0) * 512
                            c0 = 128 * (t - 4 * j)
                            nc.vector.tensor_mul(
                                et2[:, off + c0 : off + c0 + 128],
                                et2[:, off + c0 : off + c0 + 128],
                                tri,
                            )
                    ets[p0] = et2
                for u in range(4):
                    nkt = 4 * j + u + 1
                    av = pp.tile([128, 512], F32, tag="op", bufs=3)
                    for t in range(nkt):
                        sl = ets[t - t % 2][
                            :, (t % 2) * 512 + u * 128 : (t % 2) * 512 + (u + 1) * 128
                        ]
                        nc.tensor.matmul(
                            av[:, 0:129],
                            lhsT=sl,
                            rhs=vaug[b][t][:, 0:129],
                            start=(t == 0),
                            stop=(t == nkt - 1),
                        )
                    rec = ep.tile([128, 1], F32, tag="rec", bufs=2)
                    nc.vector.reciprocal(rec, av[:, 128:129])
                    ao = ep.tile([128, 128], BF16, tag="ao", bufs=2)
                    nc.vector.tensor_scalar_mul(ao, av[:, 0:128], rec)
                    pst = pp.tile([128, 128], BF16, tag="tr", bufs=1)
                    nc.tensor.matmul(pst, lhsT=ao, rhs=idbf, is_transpose=True)
                    tok0 = b * S + j * 512 + u * 128
                    nc.vector.tensor_copy(aoT[hh][:, tok0 : tok0 + 128], pst)
                    if u_cb is not None:
                        u_cb()
                if j_cb is not None:
                    j_cb(j)

        # ---------------- phase 1: qkv projection (SBUF-resident) -------
        with tc.tile_pool(name="p1", bufs=1) as p1:
            wqm = []
            for m in range(NM1):
                w_ = p1.tile([128, ET1, 128], BF16, tag=f"wq{m}", name=f"wq{m}")
                wqm.append(w_)
            cos_t = p1.tile([128, S], BF16, tag="cos")
            sin_t = p1.tile([128, S], BF16, tag="sin")
            tabs["cos"] = cos_t
            tabs["sin"] = sin_t
            # chunked first loads so the first matmul starts ~1us in
            nc.sync.dma_start(wqm[0][:, 0:8, :], h["wqkvT"][0][:, 0:8, :])
            xb0 = p1.tile([128, ET1, TB], BF16, tag="xb", bufs=2)
            nc.sync.dma_start(xb0[:, 0:8, :], h["xT"][0][:, 0:8, :])
            for ck in range(1, 4):
                nc.sync.dma_start(
                    wqm[0][:, ck * 8 : (ck + 1) * 8, :],
                    h["wqkvT"][0][:, ck * 8 : (ck + 1) * 8, :],
                )
                nc.sync.dma_start(
                    xb0[:, ck * 8 : (ck + 1) * 8, :],
                    h["xT"][0][:, ck * 8 : (ck + 1) * 8, :],
                )
            for m in range(1, NM1):
                nc.sync.dma_start(wqm[m], h["wqkvT"][m])
            nc.sync.dma_start(tri, h["tri"])
            nc.sync.dma_start(idbf, h["idbf"])
            nc.sync.dma_start(tabs["cos"], h["cos"])
            nc.sync.dma_start(tabs["sin"], h["sin"])

            xb_hold = {0: xb0}

            def p1_block(n, m):
                if m == 0 and n > 0:
                    xb_n = p1.tile([128, ET1, TB], BF16, tag="xb", bufs=2)
                    nc.sync.dma_start(xb_n, h["xT"][n])
                    xb_hold.clear()
                    xb_hold[n] = xb_n
                xb = xb_hold[n]
                ps = pp.tile([128, TB], F32, tag="op", bufs=3)
                for t in range(ET1):
                    nc.tensor.matmul(
                        ps,
                        lhsT=wqm[m][:, t, :],
                        rhs=xb[:, t, :],
                        start=(t == 0),
                        stop=(t == ET1 - 1),
                    )
                dst = qkv[m][:, n * TB : (n + 1) * TB]
                if m % 2 == 0:
                    nc.scalar.copy(dst, ps)
                else:
                    nc.vector.tensor_copy(dst, ps)

            for n in range(12):
                for m in range(NM1):
                    p1_block(n, m)
                if n == 7:
                    # batch-0 tokens complete: hide v-transpose + RoPE
                    # under the remaining phase-1 PE work (one slice per
                    # block so DVE never backs up the psum copies)
                    emit_vaug(0)
                    emit_rope_slice(0, 4)
                elif 8 <= n <= 11:
                    emit_rope_slice(0, n - 8)

            # remaining phase-1 groups are pumped into the exp-wait
            # bubbles of batch-0 heads h0-h2 (one group per 2 u-steps)
            p1g = iter([(n, m) for n in range(12, NB) for m in range(NM1)])
            cnt = [0]

            def pump_p1():
                cnt[0] += 1
                if cnt[0] % 2 == 0:
                    tg = next(p1g, None)
                    if tg is not None:
                        p1_block(*tg)

            for hh in range(3):
                attention_head(0, hh, u_cb=pump_p1)
            for tg in p1g:
                p1_block(*tg)
            # batch-1 k/q0 RoPE now (ScalarE-copied slices: safe deps);
            # h3 PE work covers the DVE time; vaug-b1 spread into h3
            emit_rope_slice(1, 4)
            emit_rope_slice(1, 0)

            def h3_j_cb(j):
                if j == 0:
                    emit_vaug(1)

            attention_head(0, 3, j_cb=h3_j_cb)
        # ---------------- late pool: exp tiles, w_out, store staging ----
        _p2cm = tc.tile_pool(name="p2", bufs=1)
        p2 = _p2cm.__enter__()
        for ob in range(8):
            w_ = p2.tile([128, 4, 512], BF16, tag=f"wo{ob}", name=f"wo{ob}")
            nc.sync.dma_start(w_, h["woutT"][ob])
            wo[ob] = w_

        # ---------------- phase 3 chunk emitters -------------------------
        bst_cur = [None]

        def p3_group(tt, g, dve_both=False):
            if g == 0:
                bst_new = p2.tile([128, EMBED], BF16, tag="ost", bufs=2)
                bst_cur[0] = bst_new
            bst = bst_cur[0]
            psA = pp.tile([128, 512], F32, tag="op", bufs=3)
            psB = pp.tile([128, 512], F32, tag="op", bufs=3)
            for et in range(4):
                lt = aoT[et][:, tt * 128 : (tt + 1) * 128]
                nc.tensor.matmul(
                    psA, lhsT=lt, rhs=wo[2 * g][:, et, :],
                    start=(et == 0), stop=(et == 3),
                )
                nc.tensor.matmul(
                    psB, lhsT=lt, rhs=wo[2 * g + 1][:, et, :],
                    start=(et == 0), stop=(et == 3),
                )
            if dve_both:
                nc.vector.tensor_copy(
                    bst[:, (2 * g) * 512 : (2 * g + 1) * 512], psA
                )
            else:
                nc.scalar.copy(bst[:, (2 * g) * 512 : (2 * g + 1) * 512], psA)
            nc.vector.tensor_copy(
                bst[:, (2 * g + 1) * 512 : (2 * g + 2) * 512], psB
            )
            if g == 3:
                nc.gpsimd.dma_start(
                    h["out"][tt * 128 : (tt + 1) * 128, :], bst
                )

        def p3_tt(tt):
            for g in range(4):
                p3_group(tt, g)

        # batch-1 attention with phase-3 b0-chunks pumped per u-step
        units = iter([(tt, g) for tt in range(16) for g in range(4)])

        def pump():
            tg = next(units, None)
            if tg is not None:
                p3_group(*tg, dve_both=True)

        def h0_j_cb(j):
            if j < 3:
                emit_rope_slice(1, j + 1)

        attention_head(1, 0, u_cb=pump, j_cb=h0_j_cb)
        for hh in range(1, 4):
            attention_head(1, hh, u_cb=pump)
        for tg in units:
            p3_group(*tg)
        for tt in range(16, 32):
            p3_tt(tt)
        _p2cm.__exit__(None, None, None)


def _declare(nc):
    h = {}
    h["xT"] = nc.dram_tensor("xT", [NB, 128, ET1, TB], BF16, kind="ExternalInput").ap()
    h["wqkvT"] = nc.dram_tensor("wqkvT", [NM1, 128, ET1, 128], BF16, kind="ExternalInput").ap()
    h["woutT"] = nc.dram_tensor("woutT", [8, 128, 4, 512], BF16, kind="ExternalInput").ap()
    h["cos"] = nc.dram_tensor("cosT", [128, S], BF16, kind="ExternalInput").ap()
    h["sin"] = nc.dram_tensor("sinT", [128, S], BF16, kind="ExternalInput").ap()
    h["tri"] = nc.dram_tensor("tri", [128, 128], BF16, kind="ExternalInput").ap()
    h["idbf"] = nc.dram_tensor("idbf", [128, 128], BF16, kind="ExternalInput").ap()
    h["out"] = nc.dram_tensor("out", [TOK, EMBED], BF16, kind="ExternalOutput").ap()
    return h


_CACHE = {}


def _get_nc():
    if "nc" not in _CACHE:
        nc = bacc.Bacc(None, target_bir_lowering=False, debug=False)
        h = _declare(nc)
        with tile.TileContext(nc) as tc:
            _emit(nc, tc, h)
        nc.compile()
        _CACHE["nc"] = nc
    return _CACHE["nc"]


def _prep_in_maps(x, w_qkv, w_out):
    x = np.asarray(x, dtype=np.float32)
    w_qkv = np.asarray(w_qkv, dtype=np.float32)
    w_out = np.asarray(w_out, dtype=np.float32)

    xT = x.reshape(TOK, EMBED).T  # [E, TOK]
    xT = np.ascontiguousarray(
        xT.reshape(ET1, 128, NB, TB).transpose(2, 1, 0, 3)
    ).astype(NPBF16)  # [n, p, t, c]

    # RoPE tables, half-layout; sin sign-folded: rows 0:64 = -sin (pairs
    # x1*c - x2*s), rows 64:128 = +sin (x2*c + x1*s)
    invf = ROPE_BASE ** (-np.arange(0, HEAD_DIM, 2, dtype=np.float32) / HEAD_DIM)
    ang = invf[:, None].astype(np.float64) * np.arange(S, dtype=np.float64)[None, :]
    cosT = np.concatenate([np.cos(ang), np.cos(ang)], axis=0).astype(NPBF16)
    sinT = np.concatenate([-np.sin(ang), np.sin(ang)], axis=0).astype(NPBF16)

    tri = np.triu(np.ones((128, 128), dtype=np.float32)).astype(NPBF16)
    idbf = np.eye(128, dtype=np.float32).astype(NPBF16)

    # interleaved -> half-layout permutation of the head dim, applied to the
    # q/k rows of the weight (scores are invariant to a shared permutation)
    perm = np.concatenate([np.arange(0, 128, 2), np.arange(1, 128, 2)])

    in_maps = []
    for c in range(NCORES):
        ws = w_qkv[c * ODIM : (c + 1) * ODIM].copy()
        for hb in range(5):  # 4 q heads + k
            ws[hb * 128 : (hb + 1) * 128] = ws[hb * 128 : (hb + 1) * 128][perm]
        wqkvT = ws.T.reshape(ET1, 128, NM1, 128).transpose(2, 1, 0, 3)
        wqkvT = np.ascontiguousarray(wqkvT).astype(NPBF16)  # [m, p, t, d]
        woutT = w_out[:, c * 512 : (c + 1) * 512].T  # [512, E]
        woutT = np.ascontiguousarray(
            woutT.reshape(4, 128, 8, 512).transpose(2, 1, 0, 3)
        ).astype(NPBF16)  # [ob, p, et, o]
        in_maps.append(
            {
                "xT": xT,
                "wqkvT": wqkvT,
                "woutT": woutT,
                "cosT": cosT,
                "sinT": sinT,
                "tri": tri,
                "idbf": idbf,
            }
        )
    return in_maps


def _run(inputs, trace=False):
    nc = _get_nc()
    in_maps = _prep_in_maps(inputs["x"], inputs["w_qkv"], inputs["w_out"])
    res = run_bass_kernel_spmd(nc, in_maps, list(range(NCORES)), trace=trace)
    acc = np.zeros((TOK, EMBED), dtype=np.float32)
    for r in res.results:
        acc += np.asarray(r["out"]).astype(np.float32)
    out = acc.reshape(B, S, EMBED)
    return out, res.exec_time_ns


def kernel(**inputs):
    out, _ = _run(inputs, trace=False)
    return out
